# revision 12
# baseline (speedup 1.0000x reference)
"""Trainium2 Bass kernel for a single-layer causal-attention decoder.

Key algebraic shortcut: VOCAB=5 and the model has no positional encoding,
so Q[q], K[k], V[k] depend only on the token ids x_q, x_k. The [S, S]
attention therefore collapses to prefix token counts:

    E5[a, t]  = exp(Q5[a] . K5[t] / 8)          # [5, 5]
    C[t, q]   = #{k <= q : x_k = t}             # prefix counts
    out[q, :] = sum_t C[t,q] E5[x_q,t] V5[t,:] / sum_t C[t,q] E5[x_q,t]

which is O(S*V) work instead of O(S^2*D). No S x S matrices exist at all.

Device layout (per core; data-parallel over batch, one sequence per core):
  - positions packed 16 blocks x 128 on partitions: partition (b, t) = b*5+t
    holds token t's lane for position block b -> all elementwise work is
    [80, 128] instead of [5, 2048].
  - one-hot via is_equal(x_broadcast, t-column); in-block inclusive prefix
    count via the DVE tensor_tensor_scan; cross-block carries via one tiny
    matmul with a host-packed [80, 80] lower-block mask, folded into the
    weight multiply with scalar_tensor_tensor.
  - E5 selection by token is an [80, 80]-block-diag matmul (e5blk built
    with one masked multiply); W = C * E5[x_q, :].
  - output: po [128, 4, 65] = W.T @ R_g where R[(b,t), (bb,m)] =
    [b == bb] * V5aug[t, m] (built once on gpsimd) — 4 wide matmuls cover
    all 16 position blocks; the ones column of V5aug gives the softmax
    denominator; reciprocal + multiply + one DMA per group.
Softmax max-subtraction is skipped; |scores/8| < ~2 so exp is tame and the
normalization cancels it exactly.
"""

import numpy as np

import concourse.bass as bass
import concourse.mybir as mybir
import concourse.tile as tile
from concourse import bacc
from concourse.bass_utils import run_bass_kernel_spmd

F32 = mybir.dt.float32
BF16 = mybir.dt.bfloat16
I32 = mybir.dt.int32

B = 8
S = 2048
D = 64
V = 5
P = 128
N_CORES = 8
MODE = "bf16"


def _np(S):
    return V * (S // P)  # packed partitions: (block b, token t) -> b*V + t


def _cbw(S):
    # wq | wk | wv_aug | etT80 | io | Lmask | blkmask | hmask | x
    return 3 * D + 1 + _np(S) + 1 + 2 * _np(S) + (S // P) + P


def _body(tc, aps, S):
    nc = tc.nc
    cb, out = aps["cb"], aps["out"]
    KB = S // P          # position blocks (= 16 at S=2048)
    NP = V * KB          # packed partitions (= 80)
    CBP = max(D + 1, NP)
    Exp = mybir.ActivationFunctionType.Exp
    add = mybir.AluOpType.add
    mult = mybir.AluOpType.mult
    bypass = mybir.AluOpType.bypass
    is_equal = mybir.AluOpType.is_equal

    from contextlib import ExitStack
    with ExitStack() as ctx:
        consts = ctx.enter_context(tc.tile_pool(name="consts", bufs=1))
        ps = ctx.enter_context(tc.tile_pool(name="ps", bufs=1, space="PSUM"))

        # ---- single bf16 constants+x DMA ----
        cb_sb = consts.tile([CBP, _cbw(S)], BF16)
        nc.sync.dma_start(cb_sb[:], cb[:])
        c = [0]
        def col(n):
            s = slice(c[0], c[0] + n)
            c[0] += n
            return s
        wqa = cb_sb[0 : D + 1, col(D)]
        wka = cb_sb[0 : D + 1, col(D)]
        wva = cb_sb[0 : D + 1, col(D + 1)]
        etT80 = cb_sb[0 : D + 1, col(NP)]   # etT tiled KB times: col (b,t) = [emb[t]; 1]
        io = cb_sb[0:NP, col(1)]            # t = p % V
        lmask = cb_sb[0:NP, col(NP)]
        bmask = cb_sb[0:NP, col(2 * NP - NP)]
        hmask = cb_sb[0:NP, col(KB)]
        xb = cb_sb[0:NP, col(P)]            # x values 0..4, exact in bf16

        # ---- one-hot + in-block prefix counts first on DVE ----
        oh = consts.tile([NP, P], BF16)
        nc.vector.tensor_tensor(oh[:], xb, io.to_broadcast((NP, P)), is_equal)
        # in-block counts <= 128: exact in bf16
        cnt = consts.tile([NP, P], BF16)
        nc.vector.tensor_tensor_scan(cnt[:], oh[:], oh[:], 0.0, add, bypass)

        # ---- tiny projected tables (all-bf16 matmuls, single-pass on PE) ----
        # V5aug80 [(b,t), m] = V5aug[t, m]; ones column from wva's e_D column
        pvv = ps.tile([NP, D + 1], F32, tag="sm", bufs=2)
        nc.tensor.matmul(pvv[:], lhsT=etT80, rhs=wva, start=True, stop=True)
        v5aug = consts.tile([NP, D + 1], BF16)
        nc.scalar.copy(v5aug[:], pvv[:])
        # Q5T tiled KB times along columns via the tiled etT80 rhs
        pq = ps.tile([D, NP], F32, tag="sm", bufs=2)
        nc.tensor.matmul(pq[:], lhsT=wqa, rhs=etT80, start=True, stop=True)
        q5t = consts.tile([D, NP], BF16)
        nc.vector.tensor_copy(q5t[:], pq[:])
        pk = ps.tile([D, V], F32, tag="sm", bufs=2)
        nc.tensor.matmul(pk[:], lhsT=wka, rhs=etT80[:, 0:V], start=True, stop=True)
        k5t = consts.tile([D, V], BF16)
        nc.scalar.copy(k5t[:], pk[:])

        # ---- R [(b,t), (bb, m)] = [b == bb] * V5aug[t, m] on gpsimd (idle
        # engine), one chunk per output group so group g never waits on g+1 ----
        G4 = min(4, KB)
        NG = KB // G4
        rsel = consts.tile([NP, KB, D + 1], BF16)
        for g in range(NG):
            gs = slice(g * G4, (g + 1) * G4)
            nc.gpsimd.tensor_tensor(
                rsel[:, gs, :],
                hmask[:, gs, None].to_broadcast((NP, G4, D + 1)),
                v5aug[:, None, :].to_broadcast((NP, G4, D + 1)),
                mult,
            )

        # ---- E5 = exp(Q5 K5.T / 8) tiled to rows (b,a); block-diag e5blk ----
        ps5 = ps.tile([NP, V], F32, tag="sm", bufs=2)
        nc.tensor.matmul(ps5[:], lhsT=q5t[:], rhs=k5t[:], start=True, stop=True)
        e5r = consts.tile([NP, V], BF16)
        nc.scalar.activation(e5r[:], ps5[:], Exp, scale=0.125)

        # ---- cross-block carries: offs[(b,t)] = sum_{b'<b} total[(b',t)] ----
        poffs = ps.tile([NP, 1], F32, tag="sm", bufs=2)
        nc.tensor.matmul(poffs[:], lhsT=lmask, rhs=cnt[:, P - 1 : P], start=True, stop=True)
        offs = consts.tile([NP, 1], F32)
        nc.vector.tensor_copy(offs[:], poffs[:])
        e5blk = consts.tile([NP, NP], BF16)
        nc.vector.tensor_tensor(
            e5blk[:].rearrange("p (b t) -> p b t", t=V),
            bmask.rearrange("p (b t) -> p b t", t=V),
            e5r[:, None, :].to_broadcast((NP, KB, V)),
            mult,
        )

        # ---- G = E5[x_q, :] via block-diag matmul; W = (cnt + offs) * G ----
        ppg = ps.tile([NP, P], F32, tag="pg", bufs=1)
        nc.tensor.matmul(ppg[:], lhsT=e5blk[:], rhs=oh[:], start=True, stop=True)
        w = consts.tile([NP, P], BF16)
        nc.vector.scalar_tensor_tensor(w[:], cnt[:], offs[:], ppg[:], add, mult)

        # ---- output: 4 wide matmuls into one 4-bank PSUM region; per-group
        # reciprocal + multiply pipeline behind the PE; one contiguous DMA ----
        # group g occupies its own bank (512 f32); its 4 blocks are packed
        # contiguously as [4, 65] so the matmul out is a flat [128, 260]
        po_all = ps.tile([P, NG, 512], F32, tag="o", bufs=1)
        rs_all = consts.tile([P, KB, D], BF16)
        for g in range(NG):
            nc.tensor.matmul(
                po_all[:, g, 0 : G4 * (D + 1)],
                lhsT=w[:], rhs=rsel[:, g * G4 : (g + 1) * G4, :],
                start=True, stop=True,
            )
        for g in range(NG):
            pov = po_all[:, g, 0 : G4 * (D + 1)].rearrange(
                "p (b m) -> p b m", m=D + 1
            )
            rc = consts.tile([P, G4, 1], F32, tag="rc", bufs=2)
            nc.vector.reciprocal(rc[:], pov[:, :, D : D + 1])
            nc.vector.tensor_tensor(
                rs_all[:, g * G4 : (g + 1) * G4, :],
                pov[:, :, 0:D],
                rc[:].to_broadcast((P, G4, D)),
                mult,
            )
        nc.sync.dma_start(out[:], rs_all[:])


def build_nc(S=S, mode=None):
    # Bacc (not plain Bass): its compile() pass splits multi-waits off
    # matmuls — TRN2 fp32 self-loading matmuls only encode one wait
    nc = bacc.Bacc(trn_type="TRN2", target_bir_lowering=False, debug=False)
    aps = {}
    aps["cb"] = nc.dram_tensor(
        "cb", [max(D + 1, _np(S)), _cbw(S)], BF16, kind="ExternalInput"
    ).ap()
    aps["out"] = nc.dram_tensor("out", [P, S // P, D], BF16, kind="ExternalOutput").ap()
    with tile.TileContext(nc) as tc:
        _body(tc, aps, S=S)
    nc.compile()
    return nc


def make_in_maps(x, emb_table, wq, bq, wk, bk, wv, bv, S=S, n_cores=N_CORES):
    import ml_dtypes
    BF = ml_dtypes.bfloat16
    x = np.asarray(x).astype(np.int32)
    emb_table = np.asarray(emb_table, dtype=np.float32)
    NP = _np(S)
    KB = S // P
    CBP = max(D + 1, NP)

    def aug(wt, bias):
        return np.vstack(
            [np.asarray(wt, np.float32).T, np.asarray(bias, np.float32)[None, :]]
        )  # [D+1, D]

    cbuf = np.zeros((CBP, _cbw(S)), np.float32)
    c = [0]
    def col(n):
        s = slice(c[0], c[0] + n)
        c[0] += n
        return s
    cbuf[: D + 1, col(D)] = aug(wq, bq)
    cbuf[: D + 1, col(D)] = aug(wk, bk)
    wva = col(D + 1)
    cbuf[: D + 1, wva][:, :D] = aug(wv, bv)
    cbuf[D, wva.start + D] = 1.0  # e_D column -> ones column of V5aug
    etT_aug = np.vstack([emb_table.T, np.ones((1, V), np.float32)])  # [65, 5]
    cbuf[: D + 1, col(NP)] = np.tile(etT_aug, (1, KB))
    pid = np.arange(NP)
    cbuf[:NP, col(1)] = (pid % V)[:, None]
    same_t = pid[:, None] % V == pid[None, :] % V
    cbuf[:NP, col(NP)] = same_t & (pid[:, None] // V < pid[None, :] // V)
    cbuf[:NP, col(NP)] = pid[:, None] // V == pid[None, :] // V
    cbuf[:NP, col(KB)] = pid[:, None] // V == np.arange(KB)[None, :]
    xcols = col(P)

    def pack(xc):
        b = cbuf.copy()
        # partition (b, t) = b*V + t holds x[b*128:(b+1)*128]; 0..4 exact
        blocks = xc[:S].reshape(KB, 1, P)
        b[:NP, xcols] = np.broadcast_to(blocks, (KB, V, P)).reshape(NP, P)
        return np.ascontiguousarray(b.astype(BF))

    return [dict(cb=pack(x[c2])) for c2 in range(n_cores)]


_NC_CACHE = {}


def _get_nc(S=S):
    if S not in _NC_CACHE:
        _NC_CACHE[S] = build_nc(S=S)
    return _NC_CACHE[S]


def run(inputs, trace=False, **kw):
    in_maps = make_in_maps(**inputs)
    nc = _get_nc()
    res = run_bass_kernel_spmd(nc, in_maps, core_ids=list(range(N_CORES)), trace=trace, **kw)
    # device keeps [P, KB, D] (one contiguous segment per partition per DMA);
    # unshard transposes back to [S, D]
    out = np.stack([
        np.ascontiguousarray(
            res.results[c]["out"].astype(np.float32).transpose(1, 0, 2).reshape(S, D)
        )
        for c in range(N_CORES)
    ])
    return out, res


def kernel(x, emb_table, wq, bq, wk, bk, wv, bv):
    out, _ = run(dict(x=x, emb_table=emb_table, wq=wq, bq=bq, wk=wk, bk=bk,
                      wv=wv, bv=bv))
    return out
